# revision 13
# baseline (speedup 1.0000x reference)
"""GAT 2-layer neighborhood-sampled kernel on 8 Trainium2 NeuronCores.

Sharding: edges partitioned by destination node across the 8 cores.
Per-edge layer-1 source features are host-gathered + host-transposed
into a dense bf16 stream (pure sequential DMA on device); layer-2
source features are device-gathered (dma_gather) from AllGather'ed
premultiplied layer-1 outputs.

L1 per-core pipeline (chunk-level software pipeline, one chunk deep):
  pass ch: per edge-tile of chunk ch: bf16 matmuls vs W1ext produce
    h||logit PSUM [128,260]; logit cols copied (ScalarE) into a
    chunk-wide staging tile, features copied (alternating ScalarE/
    VectorE) to SBUF bf16. At pass end one chunk-wide Prelu + Exp
    writes exp(alpha) into the msg tile's denominator columns. Pass
    ch+1 runs per-tile 2x-mode DVE multiplies (head-interleaved
    broadcast) and one-hot scatter matmuls into the acc PSUM, then
    finalizes (/denom, +skip+bias, ELU) and premultiplies the chunk's
    hL1 rows by [W2ext|Wskip2] (PE transpose + matmul) so layer 2
    needs no per-edge transposes or weight matmuls.
  The h2 AllGather is split into 3 chunk-groups, each issued as soon
  as its chunks finalize; layer-2 per-edge gathers for a group are
  issued right behind its AllGather, so most gather descriptor
  generation (GpSimd) overlaps the tail of L1.
L2: gathered rows are already premultiplied: per tile only the one-hot
    s_dst matmul + 2x DVE multiply + one-hot scatter matmul; segment
    softmax batched per chunk; log_softmax with Ln batched across
    chunks (one ACT table switch).
"""

import math
from contextlib import ExitStack

import numpy as np
import ml_dtypes

BF16 = ml_dtypes.bfloat16
N_CORES = 8
P = 128
NEG_SLOPE = 0.2


# ---------------------------------------------------------------- host prep

def _balance_chunks(n_dst, deg):
    """Assign local dsts to chunks of <=128 dsts, balancing edge counts."""
    n_chunks = math.ceil(n_dst / P)
    order = np.argsort(-deg, kind="stable")
    bin_edges = np.zeros(n_chunks, dtype=np.int64)
    bin_cnt = np.zeros(n_chunks, dtype=np.int64)
    pos_of_local = np.empty(n_dst, dtype=np.int64)
    for d in order:
        cand = np.where(bin_cnt < P)[0]
        b = cand[np.argmin(bin_edges[cand])]
        pos_of_local[d] = b * P + bin_cnt[b]
        bin_cnt[b] += 1
        bin_edges[b] += deg[d]
    return pos_of_local, n_chunks


def _shard_layer(src, dst, n_dst_total):
    """Per-core edge shards with self loops and balanced chunk assignment."""
    n_dst_c = n_dst_total // N_CORES
    shards = []
    for c in range(N_CORES):
        lo, hi = c * n_dst_c, (c + 1) * n_dst_c
        m = (dst >= lo) & (dst < hi)
        es = np.concatenate([src[m], np.arange(lo, hi, dtype=np.int64)])
        ed = np.concatenate(
            [(dst[m] - lo).astype(np.int64), np.arange(n_dst_c, dtype=np.int64)])
        deg = np.bincount(ed, minlength=n_dst_c)
        pos_of_local, n_chunks = _balance_chunks(n_dst_c, deg)
        shards.append(dict(esrc=es, epos=pos_of_local[ed],
                           pos_of_local=pos_of_local,
                           n_chunks=n_chunks, n_dst_c=n_dst_c))
    return shards


def _pack_edges(sh, T):
    """Chunk-major edge order, each chunk padded to T*128 edges.

    Returns (esrc [E_P] with pad=-1, dst_in_chunk [E_P] with pad=-1).
    """
    n_chunks = sh["n_chunks"]
    E_P = n_chunks * T * P
    esrc_p = np.full(E_P, -1, dtype=np.int64)
    dic_p = np.full(E_P, -1, dtype=np.int64)
    chunk_of_edge = sh["epos"] // P
    order = np.argsort(chunk_of_edge, kind="stable")
    es, ep, co = sh["esrc"][order], sh["epos"][order], chunk_of_edge[order]
    bounds = np.searchsorted(co, np.arange(n_chunks + 1))
    for ch in range(n_chunks):
        s, e = int(bounds[ch]), int(bounds[ch + 1])
        cnt = e - s
        assert cnt <= T * P
        base = ch * T * P
        esrc_p[base:base + cnt] = es[s:e]
        dic_p[base:base + cnt] = ep[s:e] - ch * P
    return esrc_p, dic_p


def _onehot_streams(dic, n_tiles, dtype):
    """dst-in-chunk [n_tiles*128] -> (ohs, ohTs) streams.

    ohs[e, t*128+d] = (dic[t*128+e] == d); ohTs is the per-tile transpose.
    Pad entries (-1) give zero rows/columns.
    """
    d = dic.reshape(n_tiles, P)
    full = (d[:, :, None] == np.arange(P)[None, None, :])
    ohs = np.ascontiguousarray(
        full.transpose(1, 0, 2).reshape(P, n_tiles * P).astype(dtype))
    ohTs = np.ascontiguousarray(
        full.transpose(2, 0, 1).reshape(P, n_tiles * P).astype(dtype))
    return ohs, ohTs


def _wrap_idx16(idx):
    """index array (len % 128 == 0) -> dma_gather SBUF layout int16."""
    n_idx = len(idx)
    cols = max(math.ceil(n_idx / 16), 1)
    flat = np.zeros(cols * 16, dtype=np.int16)
    flat[:n_idx] = idx.astype(np.int16)
    return np.tile(flat.reshape(cols, 16).T, (8, 1))


class Prep:
    def __init__(self, inputs):
        x = np.ascontiguousarray(np.asarray(inputs["x"], np.float32))
        es1 = np.asarray(inputs["edge_src1"], np.int64)
        ed1 = np.asarray(inputs["edge_dst1"], np.int64)
        es2 = np.asarray(inputs["edge_src2"], np.int64)
        ed2 = np.asarray(inputs["edge_dst2"], np.int64)
        W1 = np.asarray(inputs["W1"], np.float32)
        a_s1 = np.asarray(inputs["att_src1"], np.float32)
        a_d1 = np.asarray(inputs["att_dst1"], np.float32)
        b1 = np.asarray(inputs["b1"], np.float32)
        Wsk1 = np.asarray(inputs["Wskip1"], np.float32)
        bsk1 = np.asarray(inputs["bskip1"], np.float32)
        W2 = np.asarray(inputs["W2"], np.float32)
        a_s2 = np.asarray(inputs["att_src2"], np.float32)
        a_d2 = np.asarray(inputs["att_dst2"], np.float32)
        b2 = np.asarray(inputs["b2"], np.float32)
        Wsk2 = np.asarray(inputs["Wskip2"], np.float32)
        bsk2 = np.asarray(inputs["bskip2"], np.float32)

        self.N0, self.IN = x.shape
        self.H, self.HID = a_s1.shape
        self.OUT = a_s2.shape[1]
        H, HID, OUT, IN = self.H, self.HID, self.OUT, self.IN
        n1_lo = int(max(ed1.max(), es2.max())) + 1
        n2_lo = int(ed2.max()) + 1
        self.N1 = max(math.ceil(n1_lo / N_CORES), 1) * N_CORES
        self.N2 = max(math.ceil(n2_lo / N_CORES), 1) * N_CORES
        if self.N0 == 100000:          # the target problem's sampled sizes
            self.N1, self.N2 = 25000, 5000

        def att_block(a, width):
            A = np.zeros((H * width, H), np.float32)
            for h in range(H):
                A[h * width:(h + 1) * width, h] = a[h]
            return A

        self.D1 = H * HID
        self.C1 = self.D1 + 2 * H
        # head-interleaved hL1 feature layout: new col j <- old col
        # (j%H)*HID + j//H  (feature (hid,h) lives at col hid*H+h)
        perm1 = np.array([(j % H) * HID + j // H for j in range(self.D1)])
        w1ext = np.concatenate(
            [W1[:, perm1], W1 @ att_block(a_s1, HID), W1 @ att_block(a_d1, HID)],
            axis=1)
        self.D2 = H * OUT
        # layer-2 premultiplied row layout (head-interleaved features):
        #   [0:D2)      h2 features, col o*H+h
        #   [D2:D2+H)   s_src2
        #   [D2+H:D2+2H) s_dst2
        #   [D2+2H:D2+2H+OUT) skip (hL1 @ Wskip2)
        #   pad to G2C cols
        perm2 = np.array([(j % H) * OUT + j // H for j in range(self.D2)])
        self.G2C = max(P * math.ceil((self.D2 + 2 * H + OUT) / P), 2 * P)
        w2e = np.concatenate(
            [W2[:, perm2], W2 @ att_block(a_s2, OUT), W2 @ att_block(a_d2, OUT),
             Wsk2, np.zeros((self.D1, self.G2C - self.D2 - 2 * H - OUT),
                            np.float32)], axis=1)
        w2e = w2e[perm1, :]            # hL1 rows are interleaved
        assert w2e.shape[1] == self.G2C

        sh1 = _shard_layer(es1, ed1, self.N1)
        sh2 = _shard_layer(es2, ed2, self.N2)
        self.n_chunks1 = sh1[0]["n_chunks"]
        self.n_chunks2 = sh2[0]["n_chunks"]
        self.n_dst1_c = sh1[0]["n_dst_c"]
        self.n_dst2_c = sh2[0]["n_dst_c"]
        NCH1, NCH2 = self.n_chunks1, self.n_chunks2

        def max_tiles(shs):
            t = 1
            for sh in shs:
                cnt = np.bincount(sh["epos"] // P, minlength=sh["n_chunks"])
                t = max(t, math.ceil(cnt.max() / P))
            return t

        self.T1 = max_tiles(sh1)
        self.n_tiles1 = NCH1 * self.T1
        self.E1P = self.n_tiles1 * P
        self.ROWS1 = NCH1 * P

        # ---- AllGather chunk-groups of the premultiplied hL1 rows.
        # Small first group so its AllGather (and the layer-2 gathers
        # behind it) start early on the serial Pool/collective chain.
        N_AG = 3 if NCH1 >= 6 else 1
        if N_AG == 3:
            g0 = max(NCH1 // 5, 1)
            g1 = (NCH1 - g0 + 1) // 2
            grp_ch = [g0, g1, NCH1 - g0 - g1]
        else:
            grp_ch = [NCH1]
        grp_ch_start = np.concatenate([[0], np.cumsum(grp_ch)])
        self.N_AG = N_AG
        self.grp_ch = grp_ch
        self.grp_ch_start = grp_ch_start
        self.RG = [g * P for g in grp_ch]               # rows per group
        # h2_my rows: per group [rows..., zero-row]
        self.H2MYROWS = self.ROWS1 + N_AG
        self.grp_my_start = np.concatenate(
            [[0], np.cumsum([r + 1 for r in self.RG])])
        self.TAB_ROWS = [N_CORES * (r + 1) for r in self.RG]
        assert max(self.TAB_ROWS) < 32768

        # node -> (group, table row)
        pos1 = [sh["pos_of_local"] for sh in sh1]
        ndc1 = self.n_dst1_c
        lut_g = np.empty(self.N1, dtype=np.int64)
        lut_r = np.empty(self.N1, dtype=np.int64)
        rg_bounds = np.concatenate([[0], np.cumsum(self.RG)])
        for j in range(N_CORES):
            r = pos1[j]
            g = np.searchsorted(rg_bounds, r, side="right") - 1
            local = r - rg_bounds[g]
            rows_g = np.array(self.RG)[g] + 1
            lut_g[j * ndc1:(j + 1) * ndc1] = g
            lut_r[j * ndc1:(j + 1) * ndc1] = j * rows_g + local
        self.zrow = [rg for rg in self.RG]   # zero row idx (core 0) per table

        # ---- layer-2 edge packing: per (chunk, group), cross-core max tiles
        cnts = np.zeros((N_CORES, NCH2, N_AG), dtype=np.int64)
        ord_edges = []
        for c in range(N_CORES):
            s2 = sh2[c]
            ch_of = s2["epos"] // P
            g_of = lut_g[s2["esrc"]]
            order = np.lexsort((g_of, ch_of))
            ord_edges.append(order)
            for ch in range(NCH2):
                m = ch_of[order] == ch
                gg = g_of[order][m]
                for g in range(N_AG):
                    cnts[c, ch, g] = int((gg == g).sum())
        t2 = np.maximum(np.ceil(cnts.max(axis=0) / P).astype(np.int64), 0)
        self.t2 = t2                                    # [NCH2, N_AG]
        self.T2ch = t2.sum(axis=1)                      # tiles per chunk
        self.T2max = int(self.T2ch.max())
        self.n_tiles2 = int(self.T2ch.sum())
        self.E2P = self.n_tiles2 * P
        # per-(ch,g) tile offset within the chunk, and chunk tile offset
        self.ch_tile_off = np.concatenate([[0], np.cumsum(self.T2ch)])
        # group-stream tile offsets for the gather idx streams
        gstream_off = np.zeros((NCH2 + 1, N_AG), dtype=np.int64)
        for g in range(N_AG):
            gstream_off[1:, g] = np.cumsum(t2[:, g])
        self.gstream_off = gstream_off

        self.sh2_pos = [sh["pos_of_local"] for sh in sh2]

        rep = {
            "w1ext": np.ascontiguousarray(w1ext.astype(BF16)),
            "wskip1": np.ascontiguousarray(np.concatenate(
                [Wsk1[:, perm1], (b1 + bsk1)[perm1][None, :]],
                axis=0).astype(BF16)),
            "w2e": np.ascontiguousarray(w2e.astype(BF16)),
            "bias2": np.ascontiguousarray(
                np.tile((b2 + bsk2)[None, :], (P, 1))),
        }
        self.in_maps = []
        for c in range(N_CORES):
            s1, s2 = sh1[c], sh2[c]
            esrc1, dic1 = _pack_edges(s1, self.T1)
            xe = np.zeros((self.E1P, IN), np.float32)
            v1 = esrc1 >= 0
            xe[v1] = x[esrc1[v1]]
            xeT = np.ascontiguousarray(xe.T.astype(BF16))
            xd = np.zeros((self.ROWS1, IN), np.float32)
            lo = c * ndc1
            xd[s1["pos_of_local"]] = x[lo:lo + ndc1]
            xdstT = np.ascontiguousarray(np.concatenate(
                [xd.T, np.ones((1, self.ROWS1), np.float32)],
                axis=0).astype(BF16))
            ohs1, ohTs1 = _onehot_streams(dic1, self.n_tiles1, BF16)

            # ---- layer-2: group-sorted per-chunk slots
            order = ord_edges[c]
            es_o = s2["esrc"][order]
            ep_o = s2["epos"][order]
            ch_o = ep_o // P
            g_o = lut_g[es_o]
            dic2 = np.full(self.E2P, -1, dtype=np.int64)
            idxg = [np.full(int(gstream_off[NCH2, g]) * P, self.zrow[g],
                            dtype=np.int64) for g in range(N_AG)]
            for ch in range(NCH2):
                for g in range(N_AG):
                    m = (ch_o == ch) & (g_o == g)
                    cnt = int(m.sum())
                    if cnt == 0:
                        continue
                    toff = (self.ch_tile_off[ch] + t2[ch, :g].sum()) * P
                    dic2[toff:toff + cnt] = ep_o[m] - ch * P
                    soff = int(gstream_off[ch, g]) * P
                    idxg[g][soff:soff + cnt] = lut_r[es_o[m]]
            ohs2, ohTs2 = _onehot_streams(dic2, self.n_tiles2, BF16)

            # dst-side gather (one idx list per table; off-group -> zero row)
            dl = [np.full(NCH2 * P, self.zrow[g], dtype=np.int64)
                  for g in range(N_AG)]
            lo2 = c * self.n_dst2_c
            nodes = np.arange(lo2, lo2 + self.n_dst2_c)
            posn = s2["pos_of_local"]
            for g in range(N_AG):
                m = lut_g[nodes] == g
                dl[g][posn[m]] = lut_r[nodes[m]]
            m = {
                "xeT1": xeT,
                "ohs1": ohs1,
                "ohTs1": ohTs1,
                "xdstT1": xdstT,
                "ohs2": ohs2,
                "ohTs2": ohTs2,
            }
            for g in range(N_AG):
                m[f"idx2g{g}"] = _wrap_idx16(idxg[g])
                m[f"idxd{g}"] = _wrap_idx16(dl[g])
            m.update(rep)
            self.in_maps.append(m)

    def unshard(self, outs):
        res = np.empty((self.N2, self.OUT), np.float32)
        for c in range(N_CORES):
            o = np.asarray(outs[c]["out"])
            lo = c * self.n_dst2_c
            res[lo:lo + self.n_dst2_c] = o[self.sh2_pos[c]]
        return res


# ------------------------------------------------------------- bass program

def build_program(pp, debug=False):
    from concourse import bacc, mybir, tile
    from concourse import library_config
    from concourse.masks import make_identity

    f32 = mybir.dt.float32
    bf16 = mybir.dt.bfloat16
    i16 = mybir.dt.int16
    Alu = mybir.AluOpType
    Act = mybir.ActivationFunctionType

    IN, D1, C1 = pp.IN, pp.D1, pp.C1
    D2, OUT, H = pp.D2, pp.OUT, pp.H
    HID = pp.HID
    G2C = pp.G2C
    T1 = pp.T1
    NCH1, NCH2 = pp.n_chunks1, pp.n_chunks2
    ROWS1 = pp.ROWS1
    N_AG = pp.N_AG
    assert IN % P == 0
    KC = IN // P
    KD = D1 // P
    CM = D1 + H                       # L1 msg cols/tile: 256 feat + 4 denom
    CM2 = D2 + H                      # L2 msg cols/tile: 188 feat + 4 denom
    S2S, S2D, SK2 = D2, D2 + H, D2 + 2 * H

    nc = bacc.Bacc(None, target_bir_lowering=True, num_devices=N_CORES)

    def din(name, shape, dt):
        return nc.dram_tensor(name, shape, dt, kind="ExternalInput")

    xeT1 = din("xeT1", [IN, pp.E1P], bf16)
    ohs1 = din("ohs1", [P, pp.E1P], bf16)
    ohTs1 = din("ohTs1", [P, pp.E1P], bf16)
    xdstT1 = din("xdstT1", [IN + 1, ROWS1], bf16)
    ohs2 = din("ohs2", [P, pp.E2P], bf16)
    ohTs2 = din("ohTs2", [P, pp.E2P], bf16)
    idx2g = [din(f"idx2g{g}", [P, max(int(pp.gstream_off[NCH2, g]) * 8, 1)],
                 i16) for g in range(N_AG)]
    idxd = [din(f"idxd{g}", [P, NCH2 * 8], i16) for g in range(N_AG)]
    w1ext = din("w1ext", [IN, C1], bf16)
    wskip1 = din("wskip1", [IN + 1, D1], bf16)
    w2e = din("w2e", [D1, G2C], bf16)
    bias2 = din("bias2", [P, OUT], f32)
    out_t = nc.dram_tensor("out", [NCH2 * P, OUT], f32, kind="ExternalOutput")

    with tile.TileContext(nc) as tc, ExitStack() as top:
        const = top.enter_context(tc.tile_pool(name="const", bufs=1))
        dram = top.enter_context(tc.tile_pool(name="dram", bufs=1, space="DRAM"))

        # ---- persistent SBUF constants
        w1_sb = [const.tile([P, C1], bf16, tag=f"w1_{k}", name=f"w1_{k}")
                 for k in range(KC)]
        for k in range(KC):
            nc.sync.dma_start(w1_sb[k][:], w1ext[k * P:(k + 1) * P, :])
        wsk1_sb = [const.tile([P, D1], bf16, tag=f"wsk1_{k}", name=f"wsk1_{k}")
                   for k in range(KC)]
        for k in range(KC):
            nc.sync.dma_start(wsk1_sb[k][:], wskip1[k * P:(k + 1) * P, :])
        wsk1_ones = const.tile([1, D1], bf16)
        nc.sync.dma_start(wsk1_ones[:], wskip1[IN:IN + 1, :])
        w2p_sb = [const.tile([P, G2C], bf16, tag=f"w2p_{k}", name=f"w2p_{k}")
                  for k in range(KD)]
        for k in range(KD):
            nc.sync.dma_start(w2p_sb[k][:], w2e[k * P:(k + 1) * P, :])
        bias2_sb = const.tile([P, OUT], f32)
        nc.sync.dma_start(bias2_sb[:], bias2[:])
        ident = const.tile([P, P], f32)
        make_identity(nc, ident[:])
        identb = const.tile([P, P], bf16)
        nc.vector.tensor_copy(out=identb[:], in_=ident[:])
        xdT_sb = [const.tile([P, ROWS1], bf16, tag=f"xdT_{k}", name=f"xdT_{k}")
                  for k in range(KC)]
        for k in range(KC):
            nc.sync.dma_start(xdT_sb[k][:], xdstT1[k * P:(k + 1) * P, :])
        xdT_ones = const.tile([1, ROWS1], bf16)
        nc.sync.dma_start(xdT_ones[:], xdstT1[IN:IN + 1, :])
        idx2g_sb = [const.tile([P, idx2g[g].shape[1]], i16, tag=f"ix{g}",
                               name=f"ix{g}") for g in range(N_AG)]
        idxd_sb = [const.tile([P, NCH2 * 8], i16, tag=f"ixd{g}",
                              name=f"ixd{g}") for g in range(N_AG)]
        for g in range(N_AG):
            nc.sync.dma_start(idx2g_sb[g][:], idx2g[g][:])
            nc.sync.dma_start(idxd_sb[g][:], idxd[g][:])

        # per-group staging (separate tensors so later-group writes carry
        # no false WAR dependency on an in-flight AllGather's read)
        h2_my = [dram.tile([pp.RG[g] + 1, G2C], bf16, tag=f"h2my{g}",
                           name=f"h2my{g}") for g in range(N_AG)]
        h2tab = [dram.tile([pp.TAB_ROWS[g], G2C], bf16, addr_space="Shared",
                           tag=f"h2tab{g}", name=f"h2tab{g}")
                 for g in range(N_AG)]

        nc.gpsimd.load_library(library_config.mlp)

        # zero rows (shipped through the AllGather; dst-side gathers
        # point off-group slots at them)
        zr_sb = const.tile([1, G2C], bf16)
        nc.vector.tensor_scalar_mul(zr_sb[:], w2p_sb[0][0:1, :], 0.0)
        for g in range(N_AG):
            nc.scalar.dma_start(
                h2_my[g][pp.RG[g]:pp.RG[g] + 1, :], zr_sb[:])

        # dummy warm-up collective: the first collective in a NEFF pays
        # a ~60us ncfw lead-in; burn it concurrently with early L1 work
        dum_in = dram.tile([1, G2C], bf16, tag="dumin", name="dumin")
        dum_out = dram.tile([N_CORES, G2C], bf16, addr_space="Shared",
                            tag="dumout", name="dumout")
        nc.scalar.dma_start(dum_in[:], zr_sb[:])
        nc.gpsimd.collective_compute(
            "AllGather", Alu.bypass,
            replica_groups=[list(range(N_CORES))],
            ins=[dum_in[:]], outs=[dum_out[:]])

        # s_dst logits for all L1 chunks (bf16 for the matmul rhs)
        sdst_sb = const.tile([P, NCH1 * H], bf16)

        # early-gather output tiles (filled group by group as AGs land)
        ge_t = [const.tile([P, int(pp.T2ch[ch]) * G2C], bf16, tag=f"ge{ch}",
                           name=f"ge{ch}") for ch in range(NCH2)]
        gd_t = [const.tile([P, NCH2 * G2C], bf16, tag=f"gd{g}",
                           name=f"gd{g}") for g in range(N_AG)]
        gdm = const.tile([P, NCH2 * G2C], bf16)

        def emit_group_gathers(g):
            nc.gpsimd.collective_compute(
                "AllGather", Alu.bypass,
                replica_groups=[list(range(N_CORES))],
                ins=[h2_my[g][:]], outs=[h2tab[g][:]])
            nc.gpsimd.dma_gather(
                out_ap=gd_t[g][:].rearrange("p (c d) -> p c d", d=G2C),
                in_ap=h2tab[g][:], idxs_ap=idxd_sb[g][:],
                num_idxs=NCH2 * P, num_idxs_reg=NCH2 * P, elem_size=G2C,
                single_packet=True)
            for ch in range(NCH2):
                tg = int(pp.t2[ch, g])
                if tg == 0:
                    continue
                toff = int(pp.ch_tile_off[ch] + pp.t2[ch, :g].sum())
                coff = toff - int(pp.ch_tile_off[ch])
                soff = int(pp.gstream_off[ch, g])
                nc.gpsimd.dma_gather(
                    out_ap=ge_t[ch][:, coff * G2C:(coff + tg) * G2C]
                    .rearrange("p (c d) -> p c d", d=G2C),
                    in_ap=h2tab[g][:],
                    idxs_ap=idx2g_sb[g][:, soff * 8:(soff + tg) * 8],
                    num_idxs=tg * P, num_idxs_reg=tg * P, elem_size=G2C,
                    single_packet=True)
            if g == N_AG - 1:
                nc.vector.tensor_tensor(
                    out=gdm[:], in0=gd_t[0][:], in1=gd_t[1][:], op=Alu.add)
                for gg in range(2, N_AG):
                    nc.vector.tensor_tensor(
                        out=gdm[:], in0=gdm[:], in1=gd_t[gg][:], op=Alu.add)

        # ======================= layer 1 =======================
        with ExitStack() as l1:
            stream = l1.enter_context(tc.tile_pool(name="stream", bufs=3))
            streamS = l1.enter_context(tc.tile_pool(name="streamS", bufs=3))
            hpool = l1.enter_context(tc.tile_pool(name="hpool", bufs=2))
            mpool = l1.enter_context(tc.tile_pool(name="mpool", bufs=2))
            work = l1.enter_context(tc.tile_pool(name="work", bufs=2))
            fin = l1.enter_context(tc.tile_pool(name="fin", bufs=2))
            psH = l1.enter_context(tc.tile_pool(name="psH", bufs=5, space="PSUM"))
            psAccA = l1.enter_context(
                tc.tile_pool(name="psAccA", bufs=1, space="PSUM"))
            psAccB = l1.enter_context(
                tc.tile_pool(name="psAccB", bufs=1, space="PSUM"))
            psSkip = l1.enter_context(
                tc.tile_pool(name="psSkip", bufs=1, space="PSUM"))

            # --- per-chunk dst attention logits (s_dst)
            for ch in range(NCH1):
                csl = slice(ch * P, (ch + 1) * P)
                sd_ps = psH.tile([P, H], f32, tag="h")
                for k in range(KC):
                    nc.tensor.matmul(
                        out=sd_ps[:], lhsT=xdT_sb[k][:, csl],
                        rhs=w1_sb[k][:, D1 + H:C1],
                        start=(k == 0), stop=(k == KC - 1))
                nc.scalar.copy(out=sdst_sb[:, ch * H:(ch + 1) * H], in_=sd_ps[:])

            # --- main chunk loop, software-pipelined one chunk deep
            xs_t, ohs_t, msg_t, hsb_t = {}, {}, {}, {}
            for ch in range(NCH1 + 1):
                if ch < NCH1:
                    seg = slice(ch * T1 * P, (ch + 1) * T1 * P)
                    xs0 = stream.tile([P, T1 * P], bf16, tag="xs0")
                    xs1 = stream.tile([P, T1 * P], bf16, tag="xs1")
                    ohTt = stream.tile([P, T1 * P], bf16, tag="ohTt")
                    ohst = streamS.tile([P, T1 * P], bf16, tag="ohst")
                    nc.sync.dma_start(xs0[:], xeT1[0:P, seg])
                    nc.sync.dma_start(xs1[:], xeT1[P:2 * P, seg])
                    nc.sync.dma_start(ohTt[:], ohTs1[:, seg])
                    nc.sync.dma_start(ohst[:], ohs1[:, seg])
                    xs_t[ch] = [xs0, xs1]
                    ohs_t[ch] = ohst
                    sstage = work.tile([P, T1 * H], f32, tag="sstage")
                    hsb = hpool.tile([P, T1 * D1], bf16, tag="hsb")
                    msg = mpool.tile([P, T1 * CM], bf16, tag="msg")
                    msg_t[ch] = msg
                    hsb_t[ch] = hsb
                if ch > 0:
                    acc = (psAccA if (ch - 1) % 2 == 0 else psAccB).tile(
                        [P, D1 + H], f32, tag="acc")
                    msg_p = msg_t.pop(ch - 1)
                    hsb_p = hsb_t.pop(ch - 1)
                    ohst_p = ohs_t.pop(ch - 1)

                for i in range(T1):
                    if ch < NCH1:
                        esl = slice(i * P, (i + 1) * P)
                        h_ps = psH.tile([P, CM], f32, tag="h")
                        xs = xs_t[ch]
                        for k in range(KC):
                            nc.tensor.matmul(
                                out=h_ps[:], lhsT=xs[k][:, esl],
                                rhs=w1_sb[k][:, 0:CM],
                                start=(k == 0), stop=False)
                        nc.tensor.matmul(
                            out=h_ps[:, D1:CM], lhsT=ohTt[:, esl],
                            rhs=sdst_sb[:, ch * H:(ch + 1) * H],
                            start=False, stop=True)
                        nc.scalar.copy(
                            out=sstage[:, i * H:(i + 1) * H],
                            in_=h_ps[:, D1:CM])
                        if i % 2 == 0:
                            nc.scalar.copy(
                                out=hsb[:, i * D1:(i + 1) * D1],
                                in_=h_ps[:, 0:D1])
                        else:
                            nc.vector.tensor_copy(
                                out=hsb[:, i * D1:(i + 1) * D1],
                                in_=h_ps[:, 0:D1])
                    if ch > 0:
                        if i == 0:
                            # one chunk-wide 2x-mode multiply for all tiles
                            nc.vector.tensor_tensor(
                                out=msg_p[:].rearrange(
                                    "p (t c) -> p t c", c=CM)[:, :, 0:D1]
                                .rearrange("p t (a b) -> p t a b", b=H),
                                in0=hsb_p[:].rearrange(
                                    "p (t a b) -> p t a b", a=HID, b=H),
                                in1=msg_p[:].rearrange(
                                    "p (t c) -> p t c", c=CM
                                )[:, :, None, D1:CM].broadcast_to(
                                    [P, T1, HID, H]),
                                op=Alu.mult)
                        nc.tensor.matmul(
                            out=acc[:], lhsT=ohst_p[:, i * P:(i + 1) * P],
                            rhs=msg_p[:, i * CM:(i + 1) * CM],
                            start=(i == 0), stop=(i == T1 - 1))

                if ch < NCH1:
                    # chunk-wide attention: Prelu then Exp into msg denom cols
                    alpha = work.tile([P, T1 * H], f32, tag="alpha")
                    nc.scalar.activation(
                        out=alpha[:], in_=sstage[:], func=Act.Prelu,
                        alpha=NEG_SLOPE)
                    nc.scalar.activation(
                        out=msg[:].rearrange(
                            "p (t c) -> p t c", c=CM)[:, :, D1:CM],
                        in_=alpha[:].rearrange("p (t h) -> p t h", h=H),
                        func=Act.Exp)
                if ch > 0:
                    # ---- finalize chunk ch-1: /denom, +skip+bias, ELU,
                    #      then premultiply by [W2ext|Wskip2] and store
                    pc = ch - 1
                    csl = slice(pc * P, (pc + 1) * P)
                    accP = psAccA if pc % 2 == 0 else psAccB
                    sk_ps = psSkip.tile([P, D1], f32, tag="skip")
                    for k in range(KC):
                        nc.tensor.matmul(
                            out=sk_ps[:], lhsT=xdT_sb[k][:, csl],
                            rhs=wsk1_sb[k][:],
                            start=(k == 0), stop=False)
                    nc.tensor.matmul(
                        out=sk_ps[:], lhsT=xdT_ones[:, csl], rhs=wsk1_ones[:],
                        start=False, stop=True)
                    rec = fin.tile([P, H], f32, tag="rec")
                    nc.vector.reciprocal(rec[:], acc[:, D1:D1 + H])
                    og = fin.tile([P, D1], f32, tag="og")
                    nc.vector.tensor_tensor(
                        out=og[:].rearrange("p (a b) -> p a b", b=H),
                        in0=acc[:, 0:D1].rearrange("p (a b) -> p a b", b=H),
                        in1=rec[:][:, None, :].broadcast_to([P, HID, H]),
                        op=Alu.mult)
                    v = fin.tile([P, D1], f32, tag="v")
                    nc.vector.tensor_tensor(
                        out=v[:], in0=og[:], in1=sk_ps[:], op=Alu.add)
                    pos = fin.tile([P, D1], f32, tag="pos")
                    nc.scalar.activation(out=pos[:], in_=v[:], func=Act.Relu)
                    vneg = fin.tile([P, D1], f32, tag="vneg")
                    nc.vector.tensor_scalar_min(vneg[:], v[:], 0.0)
                    em = fin.tile([P, D1], f32, tag="em")
                    nc.scalar.activation(out=em[:], in_=vneg[:], func=Act.Exp)
                    elu = fin.tile([P, D1], bf16, tag="elu")
                    nc.vector.scalar_tensor_tensor(
                        out=elu[:], in0=em[:], scalar=-1.0, in1=pos[:],
                        op0=Alu.add, op1=Alu.add)
                    # premultiply: h2 = elu @ [W2e]  (PE transpose + matmul)
                    eT = []
                    for k in range(KD):
                        tp = accP.tile([P, P], bf16, tag="acc")
                        nc.tensor.transpose(
                            out=tp[:], in_=elu[:, k * P:(k + 1) * P],
                            identity=identb[:])
                        t_sb = fin.tile([P, P], bf16, tag=f"eT{k}")
                        nc.vector.tensor_copy(out=t_sb[:], in_=tp[:])
                        eT.append(t_sb)
                    h2_ps = psSkip.tile([P, G2C], f32, tag="skip")
                    for k in range(KD):
                        nc.tensor.matmul(
                            out=h2_ps[:], lhsT=eT[k][:], rhs=w2p_sb[k][:],
                            start=(k == 0), stop=(k == KD - 1))
                    h2sb = fin.tile([P, G2C], bf16, tag="h2sb")
                    nc.scalar.copy(out=h2sb[:], in_=h2_ps[:])
                    g = int(np.searchsorted(pp.grp_ch_start, pc, "right")) - 1
                    rowbase = (pc - int(pp.grp_ch_start[g])) * P
                    nc.scalar.dma_start(
                        h2_my[g][rowbase:rowbase + P, :], h2sb[:])
                    # group complete -> AllGather + layer-2 gathers
                    if pc + 1 == pp.grp_ch_start[g + 1]:
                        emit_group_gathers(g)

        # ======================= layer 2 =======================
        with ExitStack() as l2:
            stream2 = l2.enter_context(tc.tile_pool(name="stream2", bufs=3))
            work2 = l2.enter_context(tc.tile_pool(name="work2", bufs=2))
            fin2 = l2.enter_context(tc.tile_pool(name="fin2", bufs=2))
            lsm = l2.enter_context(tc.tile_pool(name="lsm", bufs=1))
            ps2S = l2.enter_context(tc.tile_pool(name="ps2S", bufs=2, space="PSUM"))
            ps2AccA = l2.enter_context(
                tc.tile_pool(name="ps2AccA", bufs=1, space="PSUM"))
            ps2AccB = l2.enter_context(
                tc.tile_pool(name="ps2AccB", bufs=1, space="PSUM"))

            shd_t, rs_t = {}, {}
            oh_t, ohT_t, msg2_t, ss_t = {}, {}, {}, {}
            for ch in range(NCH2 + 1):
                if ch < NCH2:
                    T2c = int(pp.T2ch[ch])
                    e0 = int(pp.ch_tile_off[ch]) * P
                    ohst2 = stream2.tile([P, T2c * P], bf16, tag="ohst2")
                    ohTt2 = stream2.tile([P, T2c * P], bf16, tag="ohTt2")
                    nc.sync.dma_start(ohst2[:], ohs2[:, e0:e0 + T2c * P])
                    nc.sync.dma_start(ohTt2[:], ohTs2[:, e0:e0 + T2c * P])
                    oh_t[ch], ohT_t[ch] = ohst2, ohTt2
                    msg2 = work2.tile([P, pp.T2max * CM2], bf16, tag="msg2")
                    msg2_t[ch] = msg2
                    ss_ps = ps2S.tile([P, pp.T2max * H], f32, tag="ss2")
                    ss_t[ch] = ss_ps
                    ge = ge_t[ch]
                    # s_dst per edge (one-hot matmuls), batched alpha
                    for i in range(T2c):
                        nc.tensor.matmul(
                            out=ss_ps[:, i * H:(i + 1) * H],
                            lhsT=ohTt2[:, i * P:(i + 1) * P],
                            rhs=gdm[:, ch * G2C + S2D:ch * G2C + S2D + H],
                            start=True, stop=True)
                    a2p = work2.tile([P, T2c * H], f32, tag="a2p")
                    nc.vector.tensor_tensor(
                        out=a2p[:].rearrange("p (t h) -> p t h", h=H),
                        in0=ss_ps[:, 0:T2c * H].rearrange(
                            "p (t h) -> p t h", h=H),
                        in1=ge[:].rearrange(
                            "p (t c) -> p t c", c=G2C)[:, :, S2S:S2S + H],
                        op=Alu.add)
                    a2f = work2.tile([P, T2c * H], f32, tag="a2f")
                    nc.scalar.activation(
                        out=a2f[:], in_=a2p[:], func=Act.Prelu,
                        alpha=NEG_SLOPE)
                    nc.scalar.activation(
                        out=msg2[:, 0:T2c * CM2].rearrange(
                            "p (t c) -> p t c", c=CM2)[:, :, D2:CM2],
                        in_=a2f[:].rearrange("p (t h) -> p t h", h=H),
                        func=Act.Exp)
                if ch > 0:
                    pc = ch - 1
                    T2p = int(pp.T2ch[pc])
                    acc2 = (ps2AccA if pc % 2 == 0 else ps2AccB).tile(
                        [P, CM2], f32, tag="acc2")
                    msg_p = msg2_t.pop(pc)
                    ge_p = ge_t[pc]
                    oh_p = oh_t.pop(pc)
                    nc.vector.tensor_tensor(
                        out=msg_p[:, 0:T2p * CM2].rearrange(
                            "p (t c) -> p t c", c=CM2)[:, :, 0:D2]
                        .rearrange("p t (a b) -> p t a b", b=H),
                        in0=ge_p[:].rearrange(
                            "p (t c) -> p t c", c=G2C)[:, :, 0:D2]
                        .rearrange("p t (a b) -> p t a b", b=H),
                        in1=msg_p[:, 0:T2p * CM2].rearrange(
                            "p (t c) -> p t c", c=CM2
                        )[:, :, None, D2:CM2].broadcast_to(
                            [P, T2p, OUT, H]),
                        op=Alu.mult)
                    for i in range(T2p):
                        nc.tensor.matmul(
                            out=acc2[:], lhsT=oh_p[:, i * P:(i + 1) * P],
                            rhs=msg_p[:, i * CM2:(i + 1) * CM2],
                            start=(i == 0), stop=(i == T2p - 1))
                    # ---- finalize: /(4*denom), mean heads, +skip+bias, lsm
                    den4 = fin2.tile([P, H], f32, tag="den4")
                    nc.vector.tensor_scalar_mul(den4[:], acc2[:, D2:CM2],
                                                float(H))
                    rec2 = fin2.tile([P, H], f32, tag="rec2")
                    nc.vector.reciprocal(rec2[:], den4[:])
                    m2 = fin2.tile([P, D2], f32, tag="m2")
                    nc.vector.tensor_tensor(
                        out=m2[:].rearrange("p (a b) -> p a b", b=H),
                        in0=acc2[:, 0:D2].rearrange("p (a b) -> p a b", b=H),
                        in1=rec2[:][:, None, :].broadcast_to([P, OUT, H]),
                        op=Alu.mult)
                    vv = fin2.tile([P, OUT], f32, tag="vv")
                    nc.vector.tensor_reduce(
                        out=vv[:], in_=m2[:].rearrange("p (a b) -> p a b", b=H),
                        axis=mybir.AxisListType.X, op=Alu.add)
                    v2 = fin2.tile([P, OUT], f32, tag="v2")
                    nc.vector.tensor_tensor(
                        out=v2[:], in0=vv[:],
                        in1=gdm[:, pc * G2C + SK2:pc * G2C + SK2 + OUT],
                        op=Alu.add)
                    v3 = fin2.tile([P, OUT], f32, tag="v3")
                    nc.vector.tensor_tensor(
                        out=v3[:], in0=v2[:], in1=bias2_sb[:], op=Alu.add)
                    rmax = fin2.tile([P, 1], f32, tag="rmax")
                    nc.vector.tensor_reduce(
                        out=rmax[:], in_=v3[:], axis=mybir.AxisListType.X,
                        op=Alu.max)
                    shd = lsm.tile([P, OUT], f32, tag=f"shd{pc}",
                                   name=f"shd{pc}")
                    nc.vector.tensor_scalar(
                        out=shd[:], in0=v3[:], scalar1=rmax[:, 0:1],
                        scalar2=None, op0=Alu.subtract)
                    exps = fin2.tile([P, OUT], f32, tag="exps")
                    rsum = lsm.tile([P, 1], f32, tag=f"rsum{pc}",
                                    name=f"rsum{pc}")
                    nc.scalar.activation(
                        out=exps[:], in_=shd[:], func=Act.Exp,
                        accum_out=rsum[:])
                    shd_t[pc], rs_t[pc] = shd, rsum
            # batched log-softmax tail (one Ln table switch)
            for ch in range(NCH2):
                lnv = fin2.tile([P, 1], f32, tag="lnv")
                nc.scalar.activation(out=lnv[:], in_=rs_t[ch][:], func=Act.Ln)
                res = fin2.tile([P, OUT], f32, tag="res")
                nc.vector.tensor_scalar(
                    out=res[:], in0=shd_t[ch][:], scalar1=lnv[:, 0:1],
                    scalar2=None, op0=Alu.subtract)
                nc.sync.dma_start(out_t[ch * P:(ch + 1) * P, :], res[:])

    nc.compile()
    return nc


# ---------------------------------------------------------------- entry

_CACHE = {}


def kernel(**inputs):
    from concourse.bass_utils import run_bass_kernel_spmd

    pp = Prep(inputs)
    key = (pp.T1, pp.n_chunks1, pp.n_chunks2, pp.IN, pp.OUT, pp.H,
           tuple(pp.T2ch))
    nc = _CACHE.get(key)
    if nc is None:
        nc = build_program(pp)
        _CACHE[key] = nc
    res = run_bass_kernel_spmd(nc, pp.in_maps, core_ids=list(range(N_CORES)))
    return pp.unshard(res.results)
